# revision 7
# baseline (speedup 1.0000x reference)
"""Causal multi-head attention (B=128, T=256, C=384, H=6, Dh=64) on 8 TRN2
NeuronCores, data-parallel over batch (16 batches per core, no collectives).

Layout strategy per core (v2 — pipelined):
  - host pre-transposes x to xT [b, C, T] and casts activations/weights to bf16
  - QT/KT computed as [D, T] (Dh on partitions) so scores = QT_h.T @ KT_h needs
    no on-chip transpose of Q/K; V computed as [T, D]
  - scores per head land in fp16 PSUM as three 128-col blocks
    [tq0ts0 | tq1ts1 | tq1ts0] so both causal-triangle blocks are adjacent:
    one DVE add of a constant -16384 triangle mask covers both
  - exp on Scalar with fused accum_out row sums (2 calls per head: tq0, tq1);
    reciprocal on DVE; normalization on GpSimd (idle engine, SBUF-only)
  - P transposed on the PE (bf16) into one packed PSUM bank per pair;
    single DVE evacuation [128, 768]
  - AV col-packs the head pair via tile_position; output projection consumes
    OT [D, T] directly; y stored bf16 (host casts back to fp32)
  - dedicated PSUM pools (proj 2 / scores 2 / transposes 2 / AV-out 2 banks)
    so head-pairs pipeline and the PE never idles during softmax
"""

import sys

sys.path.insert(0, "/opt/trn_rl_repo")

import numpy as np
import ml_dtypes

import concourse.bass as bass
import concourse.tile as tile
from concourse import mybir
from concourse.bass_utils import run_bass_kernel_spmd
from concourse.masks import make_identity

def split_multi_waits(nc):
    """This walrus build accepts at most one sync-wait command per
    instruction; hoist extra waits into standalone InstEventSemaphore
    instructions on the same engine queue (queue waits run in order before
    the original instruction, so semantics are preserved)."""
    ctr = [0]

    def mk(engine, wait):
        ctr[0] += 1
        return mybir.InstEventSemaphore(
            name=f"WSPLIT-{ctr[0]}",
            engine=engine,
            ins=[],
            outs=[],
            sync_info=mybir.SyncInfo(on_wait=[wait], on_update=[]),
        )

    for f in nc.m.functions:
        for blk in f.blocks:
            insts = blk.instructions
            out = []
            for inst in insts:
                si = inst.sync_info
                if si is not None and len(si.on_wait) > 1:
                    waits = list(si.on_wait)
                    for w in waits[:-1]:
                        out.append(mk(inst.engine, w))
                    inst.sync_info = mybir.SyncInfo(
                        on_wait=[waits[-1]], on_update=list(si.on_update)
                    )
                out.append(inst)
            insts[:] = out
    return nc


N_CORES = 8
B, T, C = 128, 256, 384
H, DH = 6, 64
BL = B // N_CORES  # batches per core
BF16 = mybir.dt.bfloat16
FP16 = mybir.dt.float16
FP32 = mybir.dt.float32
AFT = mybir.ActivationFunctionType
SCALE = DH**-0.5  # 0.125
MASKNEG = -16384.0  # exp(0.125 * -16384) == 0; exactly representable in fp16


def build_kernel() -> bass.Bass:
    nc = bass.Bass()
    xT = nc.dram_tensor("xT", [BL, C, T], BF16, kind="ExternalInput")
    wqt = nc.dram_tensor("wqt", [C, C], BF16, kind="ExternalInput")  # Wq.T [C, D]
    wkt = nc.dram_tensor("wkt", [C, C], BF16, kind="ExternalInput")
    wvt = nc.dram_tensor("wvt", [C, C], BF16, kind="ExternalInput")
    wot = nc.dram_tensor("wot", [C, C], BF16, kind="ExternalInput")  # Wo.T [D, C]
    y = nc.dram_tensor("y", [BL, T, C], BF16, kind="ExternalOutput")

    GB = 2  # batches per projection group (N = GB*T = 512 <= one PSUM bank fp32)
    with tile.TileContext(nc) as tc:
        with (
            tc.tile_pool(name="const", bufs=1) as const,
            tc.tile_pool(name="xp", bufs=2) as xp,
            tc.tile_pool(name="qkv", bufs=2) as qkv,
            tc.tile_pool(name="pp", bufs=3) as pp,
            tc.tile_pool(name="st", bufs=3) as st,
            tc.tile_pool(name="ptsb", bufs=3) as ptsb,
            tc.tile_pool(name="otp", bufs=2) as otp,
            tc.tile_pool(name="yp", bufs=3) as yp,
            tc.tile_pool(name="psProj", bufs=2, space="PSUM") as psProj,
            tc.tile_pool(name="psSc", bufs=2, space="PSUM") as psSc,
            tc.tile_pool(name="psPt", bufs=2, space="PSUM") as psPt,
            tc.tile_pool(name="psPo", bufs=2, space="PSUM") as psPo,
        ):
            ident = const.tile([128, 128], BF16)
            make_identity(nc, ident)
            # additive causal mask for the two diagonal blocks: 0 on/below the
            # diagonal, -16384 above (applied to raw scores pre-exp)
            maskadd = const.tile([128, 2, 128], FP32)
            nc.gpsimd.memset(maskadd, 0.0)
            nc.gpsimd.affine_select(
                out=maskadd, in_=maskadd, compare_op=mybir.AluOpType.is_ge,
                fill=MASKNEG, base=0, pattern=[[0, 2], [-1, 128]],
                channel_multiplier=1,
            )

            w_sb = {}
            for name, dram in (("wq", wqt), ("wk", wkt), ("wv", wvt), ("wo", wot)):
                w = const.tile([128, 3, C], BF16, tag=name)
                nc.sync.dma_start(out=w, in_=dram.rearrange("(k p) d -> p k d", p=128))
                w_sb[name] = w

            for g in range(BL // GB):
                # ---- load xT for GB batches: [128, k, b, T] ----
                xt = xp.tile([128, 3, GB, T], BF16)
                for bi in range(GB):
                    nc.sync.dma_start(
                        out=xt[:, :, bi, :],
                        in_=xT[g * GB + bi].rearrange("(k p) t -> p k t", p=128),
                    )

                # ---- QT/KT for both batches: [D, b, T], N = GB*T ----
                qt = qkv.tile([128, 3, GB, T], BF16, tag="qt")
                kt = qkv.tile([128, 3, GB, T], BF16, tag="kt")
                for dst, wname, use_v in ((qt, "wq", True), (kt, "wk", False)):
                    w = w_sb[wname]
                    for d in range(3):
                        ps = psProj.tile([128, GB * T], FP32, tag="proj")
                        for k in range(3):
                            nc.tensor.matmul(
                                ps,
                                lhsT=w[:, k, d * 128 : (d + 1) * 128],
                                rhs=xt[:, k, :, :],
                                start=(k == 0),
                                stop=(k == 2),
                            )
                        if use_v:
                            nc.vector.tensor_copy(dst[:, d, :, :], ps)
                        else:
                            nc.scalar.copy(dst[:, d, :, :], ps)

                # ---- V = [T, D] per batch ----
                vs = []
                for bi in range(GB):
                    v = qkv.tile([128, 2, C], BF16, tag=f"v{bi}")
                    for t2 in range(2):
                        ps = psProj.tile([128, GB * T], FP32, tag="proj")
                        for k in range(3):
                            nc.tensor.matmul(
                                ps[:, 0:C],
                                lhsT=xt[:, k, bi, t2 * 128 : (t2 + 1) * 128],
                                rhs=w_sb["wv"][:, k, :],
                                start=(k == 0),
                                stop=(k == 2),
                            )
                        if t2 == 0:
                            nc.vector.tensor_copy(v[:, t2, :], ps[:, 0:C])
                        else:
                            nc.scalar.copy(v[:, t2, :], ps[:, 0:C])
                    vs.append(v)

                for bi in range(GB):
                    b = g * GB + bi
                    v = vs[bi]
                    ot = otp.tile([128, 3, T], BF16)  # OT [D, T] per batch
                    for pair in range(3):
                        # ---- scores: fp32 psum, one bank per sub ----
                        # col blocks per sub: 0 = (tq0,ts0), 1 = (tq1,ts1),
                        # 2 = (tq1,ts0)
                        sc = [
                            psSc.tile([128, 3, 128], FP32, tag="sc",
                                      name=f"sc{s}")
                            for s in range(2)
                        ]
                        blks = ((0, 128, 0, 128), (128, 256, 128, 256),
                                (128, 256, 0, 128))
                        for j, (q0, q1, k0, k1) in enumerate(blks):
                            for s in range(2):
                                doff = s * 64
                                nc.tensor.matmul(
                                    sc[s][:, j, :],
                                    lhsT=qt[doff : doff + 64, pair, bi, q0:q1],
                                    rhs=kt[doff : doff + 64, pair, bi, k0:k1],
                                    start=True, stop=True,
                                )
                        # ---- additive causal mask on both diag blocks ----
                        for s in range(2):
                            nc.vector.tensor_add(
                                sc[s][:, 0:2, :], sc[s][:, 0:2, :], maskadd
                            )
                        # ---- exp with fused row sums ----
                        p = pp.tile([128, 2, 3, 128], BF16, tag="p")
                        sums = st.tile([128, 4], FP32, tag="sums")
                        rs = st.tile([128, 4], FP32, tag="rs")
                        for s in range(2):
                            nc.scalar.activation(
                                p[:, s, 0, :], sc[s][:, 0, :], AFT.Exp,
                                scale=SCALE, accum_out=sums[:, 2 * s : 2 * s + 1],
                            )
                            nc.scalar.activation(
                                p[:, s, 1:3, :], sc[s][:, 1:3, :], AFT.Exp,
                                scale=SCALE,
                                accum_out=sums[:, 2 * s + 1 : 2 * s + 2],
                            )
                        nc.vector.reciprocal(rs, sums)
                        # ---- normalize P on GpSimd (SBUF-only engine) ----
                        for s in range(2):
                            nc.gpsimd.tensor_scalar_mul(
                                p[:, s, 0, :], p[:, s, 0, :],
                                rs[:, 2 * s : 2 * s + 1],
                            )
                            nc.gpsimd.tensor_scalar_mul(
                                p[:, s, 1:3, :], p[:, s, 1:3, :],
                                rs[:, 2 * s + 1 : 2 * s + 2],
                            )
                        # ---- transpose P on the PE into one packed bank ----
                        # piece order per sub: (ts0,tq0), (ts0,tq1), (ts1,tq1)
                        ptp = psPt.tile([128, 2, 3, 128], BF16, tag="pt")
                        for s in range(2):
                            nc.tensor.transpose(ptp[:, s, 0, :], p[:, s, 0, :], ident)
                            nc.tensor.transpose(ptp[:, s, 1, :], p[:, s, 2, :], ident)
                            nc.tensor.transpose(ptp[:, s, 2, :], p[:, s, 1, :], ident)
                        pt = ptsb.tile([128, 2, 3, 128], BF16, tag="ptsb")
                        nc.vector.tensor_copy(pt, ptp)
                        # ---- AV: col-packed head pair ----
                        po = psPo.tile([128, T], FP32, tag="po")
                        for mm in range(3):
                            for s in range(2):
                                h = 2 * pair + s
                                doff = s * 64
                                vsl = lambda ts: v[:, ts, h * 64 : (h + 1) * 64]
                                if mm == 0:
                                    nc.tensor.matmul(
                                        po[doff : doff + 64, 0:128],
                                        lhsT=vsl(0), rhs=pt[:, s, 0, :],
                                        start=True, stop=True,
                                        tile_position=(0, doff),
                                    )
                                else:
                                    nc.tensor.matmul(
                                        po[doff : doff + 64, 128:256],
                                        lhsT=vsl(mm - 1), rhs=pt[:, s, mm, :],
                                        start=(mm == 1), stop=(mm == 2),
                                        tile_position=(0, doff),
                                    )
                        nc.scalar.copy(ot[:, pair, :], po)

                    # ---- y = OT.T @ WoT : [T, C] ----
                    for t2 in range(2):
                        ps = psProj.tile([128, GB * T], FP32, tag="proj")
                        for k in range(3):
                            nc.tensor.matmul(
                                ps[:, 0:C],
                                lhsT=ot[:, k, t2 * 128 : (t2 + 1) * 128],
                                rhs=w_sb["wo"][:, k, :],
                                start=(k == 0),
                                stop=(k == 2),
                            )
                        ys = yp.tile([128, C], BF16)
                        nc.vector.tensor_copy(ys, ps[:, 0:C])
                        nc.sync.dma_start(
                            out=y[b, t2 * 128 : (t2 + 1) * 128, :], in_=ys
                        )
    return nc


_NC = None


def _get_nc():
    global _NC
    if _NC is None:
        _NC = split_multi_waits(build_kernel())
    return _NC


def kernel(x, Wq, Wk, Wv, Wo, _trace=False):
    bf16 = ml_dtypes.bfloat16
    wq_t = np.ascontiguousarray(Wq.T).astype(bf16)
    wk_t = np.ascontiguousarray(Wk.T).astype(bf16)
    wv_t = np.ascontiguousarray(Wv.T).astype(bf16)
    wo_t = np.ascontiguousarray(Wo.T).astype(bf16)
    in_maps = []
    for i in range(N_CORES):
        xs = x[i * BL : (i + 1) * BL]  # [BL, T, C]
        xs_t = np.ascontiguousarray(xs.transpose(0, 2, 1)).astype(bf16)
        in_maps.append(
            {"xT": xs_t, "wqt": wq_t, "wkt": wk_t, "wvt": wv_t, "wot": wo_t}
        )
    res = run_bass_kernel_spmd(
        _get_nc(), in_maps, list(range(N_CORES)), trace=_trace
    )
    out = np.concatenate([r["y"] for r in res.results], axis=0)
    if _trace:
        return out.astype(np.float32), res
    return out.astype(np.float32)


# revision 10
# speedup vs baseline: 3.2488x; 3.2488x over previous
"""Causal multi-head attention (B=128, T=256, C=384, H=6, Dh=64) on 8 TRN2
NeuronCores, data-parallel over batch (16 batches per core, no collectives).

Layout strategy per core (v2 — pipelined):
  - host pre-transposes x to xT [b, C, T] and casts activations/weights to bf16
  - QT/KT computed as [D, T] (Dh on partitions) so scores = QT_h.T @ KT_h needs
    no on-chip transpose of Q/K; V computed as [T, D]
  - scores per head land in fp16 PSUM as three 128-col blocks
    [tq0ts0 | tq1ts1 | tq1ts0] so both causal-triangle blocks are adjacent:
    one DVE add of a constant -16384 triangle mask covers both
  - exp on Scalar with fused accum_out row sums (2 calls per head: tq0, tq1);
    reciprocal on DVE; normalization on GpSimd (idle engine, SBUF-only)
  - P transposed on the PE (bf16) into one packed PSUM bank per pair;
    single DVE evacuation [128, 768]
  - AV col-packs the head pair via tile_position; output projection consumes
    OT [D, T] directly; y stored bf16 (host casts back to fp32)
  - dedicated PSUM pools (proj 2 / scores 2 / transposes 2 / AV-out 2 banks)
    so head-pairs pipeline and the PE never idles during softmax
"""

import sys

sys.path.insert(0, "/opt/trn_rl_repo")

import numpy as np
import ml_dtypes

import concourse.bass as bass
import concourse.tile as tile
from concourse import mybir
from concourse.bass_utils import run_bass_kernel_spmd
from concourse.masks import make_identity

def split_multi_waits(nc):
    """This walrus build accepts at most one sync-wait command per
    instruction; hoist extra waits into standalone InstEventSemaphore
    instructions on the same engine queue (queue waits run in order before
    the original instruction, so semantics are preserved)."""
    ctr = [0]

    def mk(engine, wait):
        ctr[0] += 1
        return mybir.InstEventSemaphore(
            name=f"WSPLIT-{ctr[0]}",
            engine=engine,
            ins=[],
            outs=[],
            sync_info=mybir.SyncInfo(on_wait=[wait], on_update=[]),
        )

    for f in nc.m.functions:
        for blk in f.blocks:
            insts = blk.instructions
            out = []
            for inst in insts:
                si = inst.sync_info
                if si is not None and len(si.on_wait) > 1:
                    waits = list(si.on_wait)
                    for w in waits[:-1]:
                        out.append(mk(inst.engine, w))
                    inst.sync_info = mybir.SyncInfo(
                        on_wait=[waits[-1]], on_update=list(si.on_update)
                    )
                out.append(inst)
            insts[:] = out
    return nc


N_CORES = 8
B, T, C = 128, 256, 384
H, DH = 6, 64
BL = B // N_CORES  # batches per core
BF16 = mybir.dt.bfloat16
FP16 = mybir.dt.float16
FP32 = mybir.dt.float32
AFT = mybir.ActivationFunctionType
SCALE = DH**-0.5  # 0.125
MASKNEG = -16384.0  # exp(0.125 * -16384) == 0; exactly representable in fp16


def build_kernel() -> bass.Bass:
    nc = bass.Bass()
    xT = nc.dram_tensor("xT", [BL, C, T], BF16, kind="ExternalInput")
    wqt = nc.dram_tensor("wqt", [C, C], BF16, kind="ExternalInput")  # Wq.T [C, D]
    wkt = nc.dram_tensor("wkt", [C, C], BF16, kind="ExternalInput")
    wvt = nc.dram_tensor("wvt", [C, C], BF16, kind="ExternalInput")
    wot = nc.dram_tensor("wot", [C, C], BF16, kind="ExternalInput")  # Wo.T [D, C]
    y = nc.dram_tensor("y", [BL, T, C], BF16, kind="ExternalOutput")

    GB = 2  # batches per projection group (N = GB*T = 512 <= one PSUM bank fp32)
    with tile.TileContext(nc) as tc:
        with (
            tc.tile_pool(name="const", bufs=1) as const,
            tc.tile_pool(name="xp", bufs=2) as xp,
            tc.tile_pool(name="qkv", bufs=2) as qkv,
            tc.tile_pool(name="pp", bufs=3) as pp,
            tc.tile_pool(name="st", bufs=3) as st,
            tc.tile_pool(name="ptsb", bufs=3) as ptsb,
            tc.tile_pool(name="otp", bufs=2) as otp,
            tc.tile_pool(name="yp", bufs=3) as yp,
            tc.tile_pool(name="psProj", bufs=2, space="PSUM") as psProj,
            tc.tile_pool(name="psSc", bufs=2, space="PSUM") as psSc,
            tc.tile_pool(name="psPt", bufs=2, space="PSUM") as psPt,
            tc.tile_pool(name="psPo", bufs=2, space="PSUM") as psPo,
        ):
            ident = const.tile([128, 128], BF16)
            make_identity(nc, ident)
            # multiplicative causal masks (bf16), applied post-exp inside the
            # fused mask*P + row-sum DVE op. Block order per sub is
            # [tq0ts0 | tq1ts1 | tq1ts0]: blocks 0,1 are lower-triangle,
            # block 2 is all-ones. mtri1 covers blocks 1:3, mtri1[:,0,:] is
            # reused for block 0.
            mtri1 = const.tile([128, 2, 128], BF16)
            nc.gpsimd.memset(mtri1, 1.0)
            nc.gpsimd.affine_select(
                out=mtri1[:, 0, :], in_=mtri1[:, 0, :],
                compare_op=mybir.AluOpType.is_ge,
                fill=0.0, base=0, pattern=[[-1, 128]], channel_multiplier=1,
            )

            w_sb = {}
            for name, dram in (("wq", wqt), ("wk", wkt), ("wv", wvt), ("wo", wot)):
                w = const.tile([128, 3, C], BF16, tag=name)
                nc.sync.dma_start(out=w, in_=dram.rearrange("(k p) d -> p k d", p=128))
                w_sb[name] = w

            for g in range(BL // GB):
                # ---- load xT for GB batches: [128, k, b, T] ----
                xt = xp.tile([128, 3, GB, T], BF16)
                for bi in range(GB):
                    nc.sync.dma_start(
                        out=xt[:, :, bi, :],
                        in_=xT[g * GB + bi].rearrange("(k p) t -> p k t", p=128),
                    )

                # ---- QT/KT for both batches: [D, b, T], N = GB*T ----
                qt = qkv.tile([128, 3, GB, T], BF16, tag="qt")
                kt = qkv.tile([128, 3, GB, T], BF16, tag="kt")
                for dst, wname, use_v in ((qt, "wq", True), (kt, "wk", False)):
                    w = w_sb[wname]
                    for d in range(3):
                        ps = psProj.tile([128, GB * T], FP32, tag="proj")
                        for k in range(3):
                            nc.tensor.matmul(
                                ps,
                                lhsT=w[:, k, d * 128 : (d + 1) * 128],
                                rhs=xt[:, k, :, :],
                                start=(k == 0),
                                stop=(k == 2),
                            )
                        if use_v:
                            nc.vector.tensor_copy(dst[:, d, :, :], ps)
                        else:
                            nc.scalar.copy(dst[:, d, :, :], ps)

                # ---- V = [T, D] per batch ----
                vs = []
                for bi in range(GB):
                    v = qkv.tile([128, 2, C], BF16, tag=f"v{bi}")
                    for t2 in range(2):
                        ps = psProj.tile([128, GB * T], FP32, tag="proj")
                        for k in range(3):
                            nc.tensor.matmul(
                                ps[:, 0:C],
                                lhsT=xt[:, k, bi, t2 * 128 : (t2 + 1) * 128],
                                rhs=w_sb["wv"][:, k, :],
                                start=(k == 0),
                                stop=(k == 2),
                            )
                        if t2 == 0:
                            nc.vector.tensor_copy(v[:, t2, :], ps[:, 0:C])
                        else:
                            nc.scalar.copy(v[:, t2, :], ps[:, 0:C])
                    vs.append(v)

                for bi in range(GB):
                    b = g * GB + bi
                    v = vs[bi]
                    ot = otp.tile([128, 3, T], BF16)  # OT [D, T] per batch
                    for pair in range(3):
                        # ---- scores: fp32 psum, one bank per sub ----
                        # col blocks per sub: 0 = (tq0,ts0), 1 = (tq1,ts1),
                        # 2 = (tq1,ts0)
                        sc = [
                            psSc.tile([128, 3, 128], FP32, tag="sc",
                                      name=f"sc{s}")
                            for s in range(2)
                        ]
                        blks = ((0, 128, 0, 128), (128, 256, 128, 256),
                                (128, 256, 0, 128))
                        for j, (q0, q1, k0, k1) in enumerate(blks):
                            for s in range(2):
                                doff = s * 64
                                nc.tensor.matmul(
                                    sc[s][:, j, :],
                                    lhsT=qt[doff : doff + 64, pair, bi, q0:q1],
                                    rhs=kt[doff : doff + 64, pair, bi, k0:k1],
                                    start=True, stop=True,
                                )
                        # ---- exp, then fused causal-mask * P + row sums ----
                        p = pp.tile([128, 2, 3, 128], BF16, tag="p")
                        sums = st.tile([128, 4], FP32, tag="sums")
                        rs = st.tile([128, 4], FP32, tag="rs")
                        MUL = mybir.AluOpType.mult
                        ADD = mybir.AluOpType.add
                        for s in range(2):
                            nc.scalar.activation(
                                p[:, s, :, :], sc[s], AFT.Exp, scale=SCALE
                            )
                            nc.vector.scalar_tensor_tensor(
                                out=p[:, s, 0, :], in0=p[:, s, 0, :],
                                scalar=1.0, in1=mtri1[:, 0, :],
                                op0=MUL, op1=MUL,
                                accum_out=sums[:, 2 * s : 2 * s + 1],
                            )
                            nc.vector.scalar_tensor_tensor(
                                out=p[:, s, 1:3, :], in0=p[:, s, 1:3, :],
                                scalar=1.0, in1=mtri1,
                                op0=MUL, op1=MUL,
                                accum_out=sums[:, 2 * s + 1 : 2 * s + 2],
                            )
                        nc.vector.reciprocal(rs, sums)
                        # ---- normalize P (DVE per-partition scalar mul) ----
                        for s in range(2):
                            nc.vector.tensor_scalar_mul(
                                p[:, s, 0, :], p[:, s, 0, :],
                                rs[:, 2 * s : 2 * s + 1],
                            )
                            nc.vector.tensor_scalar_mul(
                                p[:, s, 1:3, :], p[:, s, 1:3, :],
                                rs[:, 2 * s + 1 : 2 * s + 2],
                            )
                        # ---- transpose P on the PE into one packed bank ----
                        # piece order per sub: (ts0,tq0), (ts0,tq1), (ts1,tq1)
                        ptp = psPt.tile([128, 2, 3, 128], BF16, tag="pt")
                        for s in range(2):
                            nc.tensor.transpose(ptp[:, s, 0, :], p[:, s, 0, :], ident)
                            nc.tensor.transpose(ptp[:, s, 1, :], p[:, s, 2, :], ident)
                            nc.tensor.transpose(ptp[:, s, 2, :], p[:, s, 1, :], ident)
                        pt = ptsb.tile([128, 2, 3, 128], BF16, tag="ptsb")
                        nc.vector.tensor_copy(pt, ptp)
                        # ---- AV: col-packed head pair ----
                        po = psPo.tile([128, T], FP32, tag="po")
                        for mm in range(3):
                            for s in range(2):
                                h = 2 * pair + s
                                doff = s * 64
                                vsl = lambda ts: v[:, ts, h * 64 : (h + 1) * 64]
                                if mm == 0:
                                    nc.tensor.matmul(
                                        po[doff : doff + 64, 0:128],
                                        lhsT=vsl(0), rhs=pt[:, s, 0, :],
                                        start=True, stop=True,
                                        tile_position=(0, doff),
                                    )
                                else:
                                    nc.tensor.matmul(
                                        po[doff : doff + 64, 128:256],
                                        lhsT=vsl(mm - 1), rhs=pt[:, s, mm, :],
                                        start=(mm == 1), stop=(mm == 2),
                                        tile_position=(0, doff),
                                    )
                        nc.scalar.copy(ot[:, pair, :], po)

                    # ---- y = OT.T @ WoT : [T, C] ----
                    for t2 in range(2):
                        ps = psProj.tile([128, GB * T], FP32, tag="proj")
                        for k in range(3):
                            nc.tensor.matmul(
                                ps[:, 0:C],
                                lhsT=ot[:, k, t2 * 128 : (t2 + 1) * 128],
                                rhs=w_sb["wo"][:, k, :],
                                start=(k == 0),
                                stop=(k == 2),
                            )
                        ys = yp.tile([128, C], BF16)
                        nc.vector.tensor_copy(ys, ps[:, 0:C])
                        nc.sync.dma_start(
                            out=y[b, t2 * 128 : (t2 + 1) * 128, :], in_=ys
                        )
    return nc


_NC = None


def _get_nc():
    global _NC
    if _NC is None:
        _NC = split_multi_waits(build_kernel())
    return _NC


def kernel(x, Wq, Wk, Wv, Wo, _trace=False):
    bf16 = ml_dtypes.bfloat16
    wq_t = np.ascontiguousarray(Wq.T).astype(bf16)
    wk_t = np.ascontiguousarray(Wk.T).astype(bf16)
    wv_t = np.ascontiguousarray(Wv.T).astype(bf16)
    wo_t = np.ascontiguousarray(Wo.T).astype(bf16)
    in_maps = []
    for i in range(N_CORES):
        xs = x[i * BL : (i + 1) * BL]  # [BL, T, C]
        xs_t = np.ascontiguousarray(xs.transpose(0, 2, 1)).astype(bf16)
        in_maps.append(
            {"xT": xs_t, "wqt": wq_t, "wkt": wk_t, "wvt": wv_t, "wot": wo_t}
        )
    res = run_bass_kernel_spmd(
        _get_nc(), in_maps, list(range(N_CORES)), trace=_trace
    )
    out = np.concatenate([r["y"] for r in res.results], axis=0)
    if _trace:
        return out.astype(np.float32), res
    return out.astype(np.float32)


# revision 11
# speedup vs baseline: 4.0882x; 1.2584x over previous
"""Causal multi-head attention (B=128, T=256, C=384, H=6, Dh=64) on 8 TRN2
NeuronCores, data-parallel over batch (16 batches per core, no collectives).

Layout strategy per core (v4 — software-pipelined):
  - host pre-transposes x to xT [b, C, T] and casts activations/weights to bf16
  - QT/KT computed as [D, T] (Dh on partitions) so scores = QT_h.T @ KT_h needs
    no on-chip transpose of Q/K; V computed as [T, D]
  - scores per head land in fp32 PSUM as three 128-col blocks
    [tq0ts0 | tq1ts1 | tq1ts0]
  - exp on Scalar; fused causal-mask-multiply + row-sum via one DVE
    scalar_tensor_tensor per (sub, tq-block); reciprocal + normalize on DVE
  - P transposed on the PE (bf16) into one packed PSUM bank per pair
  - AV col-packs the head pair via tile_position; output projection consumes
    OT [D, T] directly; y stored bf16 (host casts back to fp32)
  - group g+1's QK/V projection matmuls are emitted interleaved with group
    g's attention pairs so the PE always has filler work during softmax and
    the HAM clock gate stays warm
  - dedicated PSUM pools (proj 2 / scores 2 / transposes 2 / AV-out 2 banks)
"""

import sys

sys.path.insert(0, "/opt/trn_rl_repo")

import numpy as np
import ml_dtypes

import concourse.bass as bass
import concourse.tile as tile
from concourse import mybir
from concourse.bass_utils import run_bass_kernel_spmd
from concourse.masks import make_identity

def split_multi_waits(nc):
    """This walrus build accepts at most one sync-wait command per
    instruction; hoist extra waits into standalone InstEventSemaphore
    instructions on the same engine queue (queue waits run in order before
    the original instruction, so semantics are preserved)."""
    ctr = [0]

    def mk(engine, wait):
        ctr[0] += 1
        return mybir.InstEventSemaphore(
            name=f"WSPLIT-{ctr[0]}",
            engine=engine,
            ins=[],
            outs=[],
            sync_info=mybir.SyncInfo(on_wait=[wait], on_update=[]),
        )

    for f in nc.m.functions:
        for blk in f.blocks:
            insts = blk.instructions
            out = []
            for inst in insts:
                si = inst.sync_info
                if si is not None and len(si.on_wait) > 1:
                    waits = list(si.on_wait)
                    for w in waits[:-1]:
                        out.append(mk(inst.engine, w))
                    inst.sync_info = mybir.SyncInfo(
                        on_wait=[waits[-1]], on_update=list(si.on_update)
                    )
                out.append(inst)
            insts[:] = out
    return nc


N_CORES = 8
B, T, C = 128, 256, 384
H, DH = 6, 64
BL = B // N_CORES  # batches per core
GB = 2  # batches per projection group (N = GB*T = 512 <= one PSUM bank fp32)
NG = BL // GB
BF16 = mybir.dt.bfloat16
FP32 = mybir.dt.float32
AFT = mybir.ActivationFunctionType
MUL = mybir.AluOpType.mult
SCALE = DH**-0.5  # 0.125


def build_kernel() -> bass.Bass:
    nc = bass.Bass()
    xT = nc.dram_tensor("xT", [BL, C, T], BF16, kind="ExternalInput")
    wqt = nc.dram_tensor("wqt", [C, C], BF16, kind="ExternalInput")  # Wq.T [C, D]
    wkt = nc.dram_tensor("wkt", [C, C], BF16, kind="ExternalInput")
    wvt = nc.dram_tensor("wvt", [C, C], BF16, kind="ExternalInput")
    wot = nc.dram_tensor("wot", [C, C], BF16, kind="ExternalInput")  # Wo.T [D, C]
    y = nc.dram_tensor("y", [BL, T, C], BF16, kind="ExternalOutput")

    with tile.TileContext(nc) as tc:
        with (
            tc.tile_pool(name="const", bufs=1) as const,
            tc.tile_pool(name="xp", bufs=2) as xp,
            tc.tile_pool(name="qkv", bufs=2) as qkv,
            tc.tile_pool(name="pp", bufs=3) as pp,
            tc.tile_pool(name="st", bufs=3) as st,
            tc.tile_pool(name="ptsb", bufs=3) as ptsb,
            tc.tile_pool(name="otp", bufs=2) as otp,
            tc.tile_pool(name="yp", bufs=3) as yp,
            tc.tile_pool(name="psProj", bufs=2, space="PSUM") as psProj,
            tc.tile_pool(name="psSc", bufs=2, space="PSUM") as psSc,
            tc.tile_pool(name="psPt", bufs=2, space="PSUM") as psPt,
            tc.tile_pool(name="psPo", bufs=2, space="PSUM") as psPo,
        ):
            ident = const.tile([128, 128], BF16)
            make_identity(nc, ident)
            # multiplicative causal masks (bf16), applied post-exp inside the
            # fused mask*P + row-sum DVE op. Block order per sub is
            # [tq0ts0 | tq1ts1 | tq1ts0]: blocks 0,1 are lower-triangle,
            # block 2 is all-ones. mtri1 covers blocks 1:3, mtri1[:,0,:] is
            # reused for block 0.
            mtri1 = const.tile([128, 2, 128], BF16)
            nc.gpsimd.memset(mtri1, 1.0)
            nc.gpsimd.affine_select(
                out=mtri1[:, 0, :], in_=mtri1[:, 0, :],
                compare_op=mybir.AluOpType.is_ge,
                fill=0.0, base=0, pattern=[[-1, 128]], channel_multiplier=1,
            )

            w_sb = {}
            for name, dram in (("wq", wqt), ("wk", wkt), ("wv", wvt), ("wo", wot)):
                w = const.tile([128, 3, C], BF16, tag=name)
                nc.sync.dma_start(out=w, in_=dram.rearrange("(k p) d -> p k d", p=128))
                w_sb[name] = w

            def load_group(g):
                """DMA xT for group g, allocate qt/kt/v tiles."""
                xt = xp.tile([128, 3, GB, T], BF16, name=f"xt{g}")
                for bi in range(GB):
                    nc.sync.dma_start(
                        out=xt[:, :, bi, :],
                        in_=xT[g * GB + bi].rearrange("(k p) t -> p k t", p=128),
                    )
                qt = qkv.tile([128, 3, GB, T], BF16, tag="qt", name=f"qt{g}")
                kt = qkv.tile([128, 3, GB, T], BF16, tag="kt", name=f"kt{g}")
                vs = [
                    qkv.tile([128, 2, C], BF16, tag=f"v{bi}", name=f"v{g}_{bi}")
                    for bi in range(GB)
                ]
                return xt, qt, kt, vs

            def proj_emitters(xt, qt, kt, vs):
                """Closures each emitting one PSUM-chunk of the QK/V
                projections (3 accumulating matmuls + 1 evacuation)."""
                ems = []
                for dst, wname in ((qt, "wq"), (kt, "wk")):
                    for d in range(3):
                        def em(dst=dst, wname=wname, d=d):
                            ps = psProj.tile([128, GB * T], FP32, tag="proj",
                                             name="psqk")
                            for k in range(3):
                                nc.tensor.matmul(
                                    ps,
                                    lhsT=w_sb[wname][:, k, d * 128:(d + 1) * 128],
                                    rhs=xt[:, k, :, :],
                                    start=(k == 0), stop=(k == 2),
                                )
                            nc.scalar.copy(dst[:, d, :, :], ps)
                        ems.append(em)
                for bi in range(GB):
                    for t2 in range(2):
                        def em(bi=bi, t2=t2):
                            ps = psProj.tile([128, GB * T], FP32, tag="proj",
                                             name="psv")
                            for k in range(3):
                                nc.tensor.matmul(
                                    ps[:, 0:C],
                                    lhsT=xt[:, k, bi, t2 * 128:(t2 + 1) * 128],
                                    rhs=w_sb["wv"][:, k, :],
                                    start=(k == 0), stop=(k == 2),
                                )
                            nc.scalar.copy(vs[bi][:, t2, :], ps[:, 0:C])
                        ems.append(em)
                return ems

            def att_pair(qt, kt, v, bi, pair, ot):
                # ---- scores: fp32 psum, one bank per sub ----
                # col blocks per sub: 0 = (tq0,ts0), 1 = (tq1,ts1), 2 = (tq1,ts0)
                sc = [
                    psSc.tile([128, 3, 128], FP32, tag="sc", name=f"sc{s}")
                    for s in range(2)
                ]
                blks = ((0, 128, 0, 128), (128, 256, 128, 256),
                        (128, 256, 0, 128))
                for j, (q0, q1, k0, k1) in enumerate(blks):
                    for s in range(2):
                        doff = s * 64
                        nc.tensor.matmul(
                            sc[s][:, j, :],
                            lhsT=qt[doff:doff + 64, pair, bi, q0:q1],
                            rhs=kt[doff:doff + 64, pair, bi, k0:k1],
                            start=True, stop=True,
                        )
                # ---- exp, then fused causal-mask * P + row sums ----
                p = pp.tile([128, 2, 3, 128], BF16, tag="p")
                sums = st.tile([128, 4], FP32, tag="sums")
                rs = st.tile([128, 4], FP32, tag="rs")
                for s in range(2):
                    nc.scalar.activation(
                        p[:, s, :, :], sc[s], AFT.Exp, scale=SCALE
                    )
                    nc.vector.scalar_tensor_tensor(
                        out=p[:, s, 0, :], in0=p[:, s, 0, :],
                        scalar=1.0, in1=mtri1[:, 0, :],
                        op0=MUL, op1=MUL,
                        accum_out=sums[:, 2 * s:2 * s + 1],
                    )
                    nc.vector.scalar_tensor_tensor(
                        out=p[:, s, 1:3, :], in0=p[:, s, 1:3, :],
                        scalar=1.0, in1=mtri1,
                        op0=MUL, op1=MUL,
                        accum_out=sums[:, 2 * s + 1:2 * s + 2],
                    )
                nc.vector.reciprocal(rs, sums)
                # ---- normalize P (DVE per-partition scalar mul) ----
                for s in range(2):
                    nc.vector.tensor_scalar_mul(
                        p[:, s, 0, :], p[:, s, 0, :], rs[:, 2 * s:2 * s + 1]
                    )
                    nc.vector.tensor_scalar_mul(
                        p[:, s, 1:3, :], p[:, s, 1:3, :],
                        rs[:, 2 * s + 1:2 * s + 2],
                    )
                # ---- transpose P on the PE into one packed bank ----
                # piece order per sub: (ts0,tq0), (ts0,tq1), (ts1,tq1)
                ptp = psPt.tile([128, 2, 3, 128], BF16, tag="pt")
                pt = ptsb.tile([128, 2, 3, 128], BF16, tag="ptsb")
                for s in range(2):
                    nc.tensor.transpose(ptp[:, s, 0, :], p[:, s, 0, :], ident)
                    nc.tensor.transpose(ptp[:, s, 1, :], p[:, s, 2, :], ident)
                    nc.tensor.transpose(ptp[:, s, 2, :], p[:, s, 1, :], ident)
                    nc.vector.tensor_copy(pt[:, s, :, :], ptp[:, s, :, :])
                # ---- AV: col-packed head pair ----
                po = psPo.tile([128, T], FP32, tag="po")
                for mm in range(3):
                    for s in range(2):
                        h = 2 * pair + s
                        doff = s * 64
                        vsl = lambda ts: v[:, ts, h * 64:(h + 1) * 64]
                        if mm == 0:
                            nc.tensor.matmul(
                                po[doff:doff + 64, 0:128],
                                lhsT=vsl(0), rhs=pt[:, s, 0, :],
                                start=True, stop=True,
                                tile_position=(0, doff),
                            )
                        else:
                            nc.tensor.matmul(
                                po[doff:doff + 64, 128:256],
                                lhsT=vsl(mm - 1), rhs=pt[:, s, mm, :],
                                start=(mm == 1), stop=(mm == 2),
                                tile_position=(0, doff),
                            )
                nc.vector.tensor_copy(ot[:, pair, :], po)

            def emit_y(b, ot):
                for t2 in range(2):
                    ps = psProj.tile([128, GB * T], FP32, tag="proj", name="psy")
                    for k in range(3):
                        nc.tensor.matmul(
                            ps[:, 0:C],
                            lhsT=ot[:, k, t2 * 128:(t2 + 1) * 128],
                            rhs=w_sb["wo"][:, k, :],
                            start=(k == 0), stop=(k == 2),
                        )
                    ys = yp.tile([128, C], BF16)
                    nc.scalar.copy(ys, ps[:, 0:C])
                    nc.sync.dma_start(
                        out=y[b, t2 * 128:(t2 + 1) * 128, :], in_=ys
                    )

            # ---- prologue: group 0 projections up front ----
            cur = load_group(0)
            for em in proj_emitters(cur[0], cur[1], cur[2], cur[3]):
                em()

            for g in range(NG):
                nxt_ems = []
                nxt = None
                if g + 1 < NG:
                    nxt = load_group(g + 1)
                    nxt_ems = proj_emitters(nxt[0], nxt[1], nxt[2], nxt[3])
                _, qt, kt, vs = cur
                ei = 0
                for bi in range(GB):
                    b = g * GB + bi
                    ot = otp.tile([128, 3, T], BF16)
                    for pair in range(3):
                        # interleave next group's projection chunks so the PE
                        # has filler work while this pair's softmax runs
                        for _ in range(2):
                            if ei < len(nxt_ems):
                                nxt_ems[ei]()
                                ei += 1
                        att_pair(qt, kt, vs[bi], bi, pair, ot)
                    emit_y(b, ot)
                while ei < len(nxt_ems):
                    nxt_ems[ei]()
                    ei += 1
                cur = nxt
    return nc


_NC = None


def _get_nc():
    global _NC
    if _NC is None:
        _NC = split_multi_waits(build_kernel())
    return _NC


def kernel(x, Wq, Wk, Wv, Wo, _trace=False):
    bf16 = ml_dtypes.bfloat16
    wq_t = np.ascontiguousarray(Wq.T).astype(bf16)
    wk_t = np.ascontiguousarray(Wk.T).astype(bf16)
    wv_t = np.ascontiguousarray(Wv.T).astype(bf16)
    wo_t = np.ascontiguousarray(Wo.T).astype(bf16)
    in_maps = []
    for i in range(N_CORES):
        xs = x[i * BL : (i + 1) * BL]  # [BL, T, C]
        xs_t = np.ascontiguousarray(xs.transpose(0, 2, 1)).astype(bf16)
        in_maps.append(
            {"xT": xs_t, "wqt": wq_t, "wkt": wk_t, "wvt": wv_t, "wot": wo_t}
        )
    res = run_bass_kernel_spmd(
        _get_nc(), in_maps, list(range(N_CORES)), trace=_trace
    )
    out = np.concatenate([r["y"] for r in res.results], axis=0)
    if _trace:
        return out.astype(np.float32), res
    return out.astype(np.float32)


# revision 16
# speedup vs baseline: 4.4628x; 1.0916x over previous
"""Causal multi-head attention (B=128, T=256, C=384, H=6, Dh=64) on 8 TRN2
NeuronCores, data-parallel over batch (16 batches per core, no collectives).

Layout strategy per core (v4 — software-pipelined):
  - host pre-transposes x to xT [b, C, T] and casts activations/weights to bf16
  - QT/KT computed as [D, T] (Dh on partitions) so scores = QT_h.T @ KT_h needs
    no on-chip transpose of Q/K; V computed as [T, D]
  - scores per head land in fp32 PSUM as three 128-col blocks
    [tq0ts0 | tq1ts1 | tq1ts0]
  - exp on Scalar; fused causal-mask-multiply + row-sum via one DVE
    scalar_tensor_tensor per (sub, tq-block); reciprocal + normalize on DVE
  - P transposed on the PE (bf16) into one packed PSUM bank per pair
  - AV col-packs the head pair via tile_position; output projection consumes
    OT [D, T] directly; y stored bf16 (host casts back to fp32)
  - group g+1's QK/V projection matmuls are emitted interleaved with group
    g's attention pairs so the PE always has filler work during softmax and
    the HAM clock gate stays warm
  - dedicated PSUM pools (proj 2 / scores 2 / transposes 2 / AV-out 2 banks)
"""

import sys

sys.path.insert(0, "/opt/trn_rl_repo")

import numpy as np
import ml_dtypes

import concourse.bass as bass
import concourse.tile as tile
from concourse import mybir
from concourse.bass_utils import run_bass_kernel_spmd
from concourse.masks import make_identity

def split_multi_waits(nc):
    """This walrus build accepts at most one sync-wait command per
    instruction; hoist extra waits into standalone InstEventSemaphore
    instructions on the same engine queue (queue waits run in order before
    the original instruction, so semantics are preserved)."""
    ctr = [0]

    def mk(engine, wait):
        ctr[0] += 1
        return mybir.InstEventSemaphore(
            name=f"WSPLIT-{ctr[0]}",
            engine=engine,
            ins=[],
            outs=[],
            sync_info=mybir.SyncInfo(on_wait=[wait], on_update=[]),
        )

    for f in nc.m.functions:
        for blk in f.blocks:
            insts = blk.instructions
            out = []
            for inst in insts:
                si = inst.sync_info
                if si is not None and len(si.on_wait) > 1:
                    waits = list(si.on_wait)
                    for w in waits[:-1]:
                        out.append(mk(inst.engine, w))
                    inst.sync_info = mybir.SyncInfo(
                        on_wait=[waits[-1]], on_update=list(si.on_update)
                    )
                out.append(inst)
            insts[:] = out
    return nc


N_CORES = 8
B, T, C = 128, 256, 384
H, DH = 6, 64
BL = B // N_CORES  # batches per core
GB = 2  # batches per projection group (N = GB*T = 512 <= one PSUM bank fp32)
NG = BL // GB
BF16 = mybir.dt.bfloat16
FP32 = mybir.dt.float32
AFT = mybir.ActivationFunctionType
MUL = mybir.AluOpType.mult
SCALE = DH**-0.5  # 0.125


def build_kernel() -> bass.Bass:
    nc = bass.Bass()
    xT = nc.dram_tensor("xT", [BL, C, T], BF16, kind="ExternalInput")
    wqt = nc.dram_tensor("wqt", [C, C], BF16, kind="ExternalInput")  # Wq.T [C, D]
    wkt = nc.dram_tensor("wkt", [C, C], BF16, kind="ExternalInput")
    wvt = nc.dram_tensor("wvt", [C, C], BF16, kind="ExternalInput")
    wot = nc.dram_tensor("wot", [C, C], BF16, kind="ExternalInput")  # Wo.T [D, C]
    y = nc.dram_tensor("y", [BL, T, C], BF16, kind="ExternalOutput")

    with tile.TileContext(nc) as tc:
        with (
            tc.tile_pool(name="const", bufs=1) as const,
            tc.tile_pool(name="xp", bufs=2) as xp,
            tc.tile_pool(name="qkv", bufs=2) as qkv,
            tc.tile_pool(name="pp", bufs=3) as pp,
            tc.tile_pool(name="st", bufs=3) as st,
            tc.tile_pool(name="ptsb", bufs=3) as ptsb,
            tc.tile_pool(name="otp", bufs=2) as otp,
            tc.tile_pool(name="yp", bufs=3) as yp,
            tc.tile_pool(name="psProj", bufs=2, space="PSUM") as psProj,
            tc.tile_pool(name="psSc", bufs=2, space="PSUM") as psSc,
            tc.tile_pool(name="psPt", bufs=2, space="PSUM") as psPt,
            tc.tile_pool(name="psPo", bufs=2, space="PSUM") as psPo,
        ):
            ident = const.tile([128, 128], BF16)
            make_identity(nc, ident)
            # multiplicative causal masks (bf16), applied post-exp inside the
            # fused mask*P + row-sum DVE op. Block order per sub is
            # [tq0ts0 | tq1ts0 | tq1ts1]: blocks 0,2 are lower-triangle,
            # block 1 is all-ones. mtri covers blocks 1:3 (the tq1 row);
            # mtri[:,1,:] is reused for block 0.
            mtri = const.tile([128, 2, 128], BF16)
            nc.gpsimd.memset(mtri, 1.0)
            nc.gpsimd.affine_select(
                out=mtri[:, 1, :], in_=mtri[:, 1, :],
                compare_op=mybir.AluOpType.is_ge,
                fill=0.0, base=0, pattern=[[-1, 128]], channel_multiplier=1,
            )

            w_sb = {}
            for name, dram in (("wq", wqt), ("wk", wkt), ("wv", wvt), ("wo", wot)):
                w = const.tile([128, 3, C], BF16, tag=name)
                nc.sync.dma_start(out=w, in_=dram.rearrange("(k p) d -> p k d", p=128))
                w_sb[name] = w

            def load_group(g):
                """DMA xT for group g, allocate qt/kt/v tiles."""
                xt = xp.tile([128, 3, GB, T], BF16, name=f"xt{g}")
                for bi in range(GB):
                    nc.sync.dma_start(
                        out=xt[:, :, bi, :],
                        in_=xT[g * GB + bi].rearrange("(k p) t -> p k t", p=128),
                    )
                qt = qkv.tile([128, 3, GB, T], BF16, tag="qt", name=f"qt{g}")
                kt = qkv.tile([128, 3, GB, T], BF16, tag="kt", name=f"kt{g}")
                vs = [
                    qkv.tile([128, 2, C], BF16, tag=f"v{bi}", name=f"v{g}_{bi}")
                    for bi in range(GB)
                ]
                return xt, qt, kt, vs

            def proj_emitters(xt, qt, kt, vs):
                """Closures each emitting one PSUM-chunk of the QK/V
                projections (3 accumulating matmuls + 1 evacuation)."""
                ems = []
                for dst, wname in ((qt, "wq"), (kt, "wk")):
                    for d in range(3):
                        def em(dst=dst, wname=wname, d=d):
                            ps = psProj.tile([128, GB * T], FP32, tag="proj",
                                             name="psqk")
                            for k in range(3):
                                nc.tensor.matmul(
                                    ps,
                                    lhsT=w_sb[wname][:, k, d * 128:(d + 1) * 128],
                                    rhs=xt[:, k, :, :],
                                    start=(k == 0), stop=(k == 2),
                                )
                            nc.scalar.copy(dst[:, d, :, :], ps)
                        ems.append(em)
                for bi in range(GB):
                    for t2 in range(2):
                        def em(bi=bi, t2=t2):
                            ps = psProj.tile([128, GB * T], FP32, tag="proj",
                                             name="psv")
                            for k in range(3):
                                nc.tensor.matmul(
                                    ps[:, 0:C],
                                    lhsT=xt[:, k, bi, t2 * 128:(t2 + 1) * 128],
                                    rhs=w_sb["wv"][:, k, :],
                                    start=(k == 0), stop=(k == 2),
                                )
                            nc.scalar.copy(vs[bi][:, t2, :], ps[:, 0:C])
                        ems.append(em)
                return ems

            def att_pair(qt, kt, v, bi, pair, ot, filler=None):
                # ---- scores: fp32 psum, one bank per sub ----
                # col blocks per sub: 0 = (tq0,ts0), 1 = (tq1,ts0), 2 = (tq1,ts1)
                sc = [
                    psSc.tile([128, 3, 128], FP32, tag="sc", name=f"sc{s}")
                    for s in range(2)
                ]
                for s in range(2):
                    doff = s * 64
                    nc.tensor.matmul(
                        sc[s][:, 0, :],
                        lhsT=qt[doff:doff + 64, pair, bi, 0:128],
                        rhs=kt[doff:doff + 64, pair, bi, 0:128],
                        start=True, stop=True,
                    )
                for s in range(2):
                    doff = s * 64
                    nc.tensor.matmul(
                        sc[s][:, 1:3, :],
                        lhsT=qt[doff:doff + 64, pair, bi, 128:256],
                        rhs=kt[doff:doff + 64, pair, bi, 0:256],
                        start=True, stop=True,
                    )
                # ---- exp, then fused causal-mask * P + row sums ----
                p = pp.tile([128, 2, 3, 128], BF16, tag="p")
                sums = st.tile([128, 4], FP32, tag="sums")
                rs = st.tile([128, 4], FP32, tag="rs")
                for s in range(2):
                    nc.scalar.activation(
                        p[:, s, :, :], sc[s], AFT.Exp, scale=SCALE
                    )
                    nc.vector.scalar_tensor_tensor(
                        out=p[:, s, 0, :], in0=p[:, s, 0, :],
                        scalar=1.0, in1=mtri[:, 1, :],
                        op0=MUL, op1=MUL,
                        accum_out=sums[:, 2 * s:2 * s + 1],
                    )
                    nc.vector.scalar_tensor_tensor(
                        out=p[:, s, 1:3, :], in0=p[:, s, 1:3, :],
                        scalar=1.0, in1=mtri,
                        op0=MUL, op1=MUL,
                        accum_out=sums[:, 2 * s + 1:2 * s + 2],
                    )
                nc.vector.reciprocal(rs, sums)
                # ---- normalize P (DVE per-partition scalar mul) ----
                for s in range(2):
                    nc.vector.tensor_scalar_mul(
                        p[:, s, 0, :], p[:, s, 0, :], rs[:, 2 * s:2 * s + 1]
                    )
                    nc.vector.tensor_scalar_mul(
                        p[:, s, 1:3, :], p[:, s, 1:3, :],
                        rs[:, 2 * s + 1:2 * s + 2],
                    )
                if filler is not None:
                    filler()
                # ---- transpose P on the PE into one packed bank ----
                # piece order per sub: (ts0,tq0), (ts0,tq1), (ts1,tq1)
                ptp = psPt.tile([128, 2, 3, 128], BF16, tag="pt")
                pt = ptsb.tile([128, 2, 3, 128], BF16, tag="ptsb")
                for s in range(2):
                    nc.tensor.transpose(ptp[:, s, 0, :], p[:, s, 0, :], ident)
                    nc.tensor.transpose(ptp[:, s, 1, :], p[:, s, 1, :], ident)
                    nc.tensor.transpose(ptp[:, s, 2, :], p[:, s, 2, :], ident)
                nc.vector.tensor_copy(pt, ptp)
                # ---- AV: col-packed head pair ----
                po = psPo.tile([128, T], FP32, tag="po")
                for mm in range(3):
                    for s in range(2):
                        h = 2 * pair + s
                        doff = s * 64
                        vsl = lambda ts: v[:, ts, h * 64:(h + 1) * 64]
                        if mm == 0:
                            nc.tensor.matmul(
                                po[doff:doff + 64, 0:128],
                                lhsT=vsl(0), rhs=pt[:, s, 0, :],
                                start=True, stop=True,
                                tile_position=(0, doff),
                            )
                        else:
                            nc.tensor.matmul(
                                po[doff:doff + 64, 128:256],
                                lhsT=vsl(mm - 1), rhs=pt[:, s, mm, :],
                                start=(mm == 1), stop=(mm == 2),
                                tile_position=(0, doff),
                            )
                nc.vector.tensor_copy(ot[:, pair, :], po)

            def emit_y(b, ot):
                ys = yp.tile([128, 2, C], BF16)
                for t2 in range(2):
                    ps = psProj.tile([128, GB * T], FP32, tag="proj", name="psy")
                    for k in range(3):
                        nc.tensor.matmul(
                            ps[:, 0:C],
                            lhsT=ot[:, k, t2 * 128:(t2 + 1) * 128],
                            rhs=w_sb["wo"][:, k, :],
                            start=(k == 0), stop=(k == 2),
                        )
                    nc.scalar.copy(ys[:, t2, :], ps[:, 0:C])
                nc.sync.dma_start(
                    out=y[b].rearrange("(t2 p) c -> p t2 c", p=128), in_=ys
                )

            # ---- prologue: group 0 projections up front ----
            cur = load_group(0)
            for em in proj_emitters(cur[0], cur[1], cur[2], cur[3]):
                em()

            for g in range(NG):
                nxt_ems = []
                nxt = None
                if g + 1 < NG:
                    nxt = load_group(g + 1)
                    nxt_ems = proj_emitters(nxt[0], nxt[1], nxt[2], nxt[3])
                _, qt, kt, vs = cur
                ei = [0]

                def filler():
                    # one projection chunk of the next group, emitted between
                    # softmax and transposes so the PE has work during the
                    # DVE stages
                    if ei[0] < len(nxt_ems):
                        nxt_ems[ei[0]]()
                        ei[0] += 1

                for bi in range(GB):
                    b = g * GB + bi
                    ot = otp.tile([128, 3, T], BF16)
                    for pair in range(3):
                        filler()
                        att_pair(qt, kt, vs[bi], bi, pair, ot, filler=filler)
                    emit_y(b, ot)
                while ei[0] < len(nxt_ems):
                    nxt_ems[ei[0]]()
                    ei[0] += 1
                cur = nxt
    return nc


_NC = None


def _get_nc():
    global _NC
    if _NC is None:
        _NC = split_multi_waits(build_kernel())
    return _NC


def kernel(x, Wq, Wk, Wv, Wo, _trace=False):
    bf16 = ml_dtypes.bfloat16
    wq_t = np.ascontiguousarray(Wq.T).astype(bf16)
    wk_t = np.ascontiguousarray(Wk.T).astype(bf16)
    wv_t = np.ascontiguousarray(Wv.T).astype(bf16)
    wo_t = np.ascontiguousarray(Wo.T).astype(bf16)
    in_maps = []
    for i in range(N_CORES):
        xs = x[i * BL : (i + 1) * BL]  # [BL, T, C]
        xs_t = np.ascontiguousarray(xs.transpose(0, 2, 1)).astype(bf16)
        in_maps.append(
            {"xT": xs_t, "wqt": wq_t, "wkt": wk_t, "wvt": wv_t, "wot": wo_t}
        )
    res = run_bass_kernel_spmd(
        _get_nc(), in_maps, list(range(N_CORES)), trace=_trace
    )
    out = np.concatenate([r["y"] for r in res.results], axis=0)
    if _trace:
        return out.astype(np.float32), res
    return out.astype(np.float32)


# revision 20
# speedup vs baseline: 4.5370x; 1.0166x over previous
"""Causal multi-head attention (B=128, T=256, C=384, H=6, Dh=64) on 8 TRN2
NeuronCores, data-parallel over batch (16 batches per core, no collectives).

Layout strategy per core (v4 — software-pipelined):
  - host pre-transposes x to xT [b, C, T] and casts activations/weights to bf16
  - QT/KT computed as [D, T] (Dh on partitions) so scores = QT_h.T @ KT_h needs
    no on-chip transpose of Q/K; V computed as [T, D]
  - scores per head land in fp32 PSUM as three 128-col blocks
    [tq0ts0 | tq1ts1 | tq1ts0]
  - exp on Scalar; fused causal-mask-multiply + row-sum via one DVE
    scalar_tensor_tensor per (sub, tq-block); reciprocal + normalize on DVE
  - P transposed on the PE (bf16) into one packed PSUM bank per pair
  - AV col-packs the head pair via tile_position; output projection consumes
    OT [D, T] directly; y stored bf16 (host casts back to fp32)
  - group g+1's QK/V projection matmuls are emitted interleaved with group
    g's attention pairs so the PE always has filler work during softmax and
    the HAM clock gate stays warm
  - dedicated PSUM pools (proj 2 / scores 2 / transposes 2 / AV-out 2 banks)
"""

import sys

sys.path.insert(0, "/opt/trn_rl_repo")

import numpy as np
import ml_dtypes

import concourse.bass as bass
import concourse.tile as tile
from concourse import mybir
from concourse.bass_utils import run_bass_kernel_spmd
from concourse.masks import make_identity

def split_multi_waits(nc):
    """This walrus build accepts at most one sync-wait command per
    instruction; hoist extra waits into standalone InstEventSemaphore
    instructions on the same engine queue (queue waits run in order before
    the original instruction, so semantics are preserved)."""
    ctr = [0]

    def mk(engine, wait):
        ctr[0] += 1
        return mybir.InstEventSemaphore(
            name=f"WSPLIT-{ctr[0]}",
            engine=engine,
            ins=[],
            outs=[],
            sync_info=mybir.SyncInfo(on_wait=[wait], on_update=[]),
        )

    for f in nc.m.functions:
        for blk in f.blocks:
            insts = blk.instructions
            out = []
            for inst in insts:
                si = inst.sync_info
                if si is not None and len(si.on_wait) > 1:
                    waits = list(si.on_wait)
                    for w in waits[:-1]:
                        out.append(mk(inst.engine, w))
                    inst.sync_info = mybir.SyncInfo(
                        on_wait=[waits[-1]], on_update=list(si.on_update)
                    )
                out.append(inst)
            insts[:] = out
    return nc


N_CORES = 8
B, T, C = 128, 256, 384
H, DH = 6, 64
BL = B // N_CORES  # batches per core
GB = 2  # batches per projection group (N = GB*T = 512 <= one PSUM bank fp32)
NG = BL // GB
BF16 = mybir.dt.bfloat16
FP32 = mybir.dt.float32
AFT = mybir.ActivationFunctionType
MUL = mybir.AluOpType.mult
SCALE = DH**-0.5  # 0.125


def build_kernel() -> bass.Bass:
    nc = bass.Bass()
    xT = nc.dram_tensor("xT", [BL, C, T], BF16, kind="ExternalInput")
    wqt = nc.dram_tensor("wqt", [C, C], BF16, kind="ExternalInput")  # Wq.T [C, D]
    wkt = nc.dram_tensor("wkt", [C, C], BF16, kind="ExternalInput")
    wvt = nc.dram_tensor("wvt", [C, C], BF16, kind="ExternalInput")
    wot = nc.dram_tensor("wot", [C, C], BF16, kind="ExternalInput")  # Wo.T [D, C]
    y = nc.dram_tensor("y", [BL, T, C], BF16, kind="ExternalOutput")

    with tile.TileContext(nc) as tc:
        with (
            tc.tile_pool(name="const", bufs=1) as const,
            tc.tile_pool(name="xp", bufs=2) as xp,
            tc.tile_pool(name="qkv", bufs=2) as qkv,
            tc.tile_pool(name="pp", bufs=3) as pp,
            tc.tile_pool(name="st", bufs=3) as st,
            tc.tile_pool(name="ptsb", bufs=3) as ptsb,
            tc.tile_pool(name="otp", bufs=2) as otp,
            tc.tile_pool(name="yp", bufs=3) as yp,
            tc.tile_pool(name="psProj", bufs=2, space="PSUM") as psProj,
            tc.tile_pool(name="psSc", bufs=2, space="PSUM") as psSc,
            tc.tile_pool(name="psPt", bufs=2, space="PSUM") as psPt,
            tc.tile_pool(name="psPo", bufs=2, space="PSUM") as psPo,
        ):
            # prefetch x for group 0 ahead of the (larger) weight DMAs so the
            # first projections start ASAP
            xt0 = xp.tile([128, 3, GB, T], BF16, name="xt_g0")
            for bi in range(GB):
                nc.sync.dma_start(
                    out=xt0[:, :, bi, :],
                    in_=xT[bi].rearrange("(k p) t -> p k t", p=128),
                )
            ident = const.tile([128, 128], BF16)
            make_identity(nc, ident)
            # tiny dummy exp: forces the ACT exp-table load during the DMA
            # wait instead of on the first real softmax
            dummy = const.tile([128, 2], FP32)
            nc.scalar.activation(dummy, ident[:, 0:2], AFT.Exp, scale=1.0)
            # multiplicative causal masks (bf16), applied post-exp inside the
            # fused mask*P + row-sum DVE op. Block order per sub is
            # [tq0ts0 | tq1ts0 | tq1ts1]: blocks 0,2 are lower-triangle,
            # block 1 is all-ones. mtri covers blocks 1:3 (the tq1 row);
            # mtri[:,1,:] is reused for block 0.
            mtri = const.tile([128, 2, 128], BF16)
            nc.gpsimd.memset(mtri, 1.0)
            nc.gpsimd.affine_select(
                out=mtri[:, 1, :], in_=mtri[:, 1, :],
                compare_op=mybir.AluOpType.is_ge,
                fill=0.0, base=0, pattern=[[-1, 128]], channel_multiplier=1,
            )

            w_sb = {}
            for name, dram in (("wq", wqt), ("wk", wkt), ("wv", wvt), ("wo", wot)):
                w = const.tile([128, 3, C], BF16, tag=name)
                nc.sync.dma_start(out=w, in_=dram.rearrange("(k p) d -> p k d", p=128))
                w_sb[name] = w

            def load_group(g, xt=None):
                """DMA xT for group g, allocate qt/kt/v tiles."""
                if xt is None:
                    xt = xp.tile([128, 3, GB, T], BF16, name=f"xt{g}")
                    for bi in range(GB):
                        nc.sync.dma_start(
                            out=xt[:, :, bi, :],
                            in_=xT[g * GB + bi].rearrange(
                                "(k p) t -> p k t", p=128
                            ),
                        )
                qt = qkv.tile([128, 3, GB, T], BF16, tag="qt", name=f"qt{g}")
                kt = qkv.tile([128, 3, GB, T], BF16, tag="kt", name=f"kt{g}")
                vs = [
                    qkv.tile([128, 2, C], BF16, tag=f"v{bi}", name=f"v{g}_{bi}")
                    for bi in range(GB)
                ]
                return xt, qt, kt, vs

            def proj_emitters(xt, qt, kt, vs):
                """Closures each emitting one PSUM-chunk of the QK/V
                projections (3 accumulating matmuls + 1 evacuation). Ordered
                so the consumers' dependencies resolve earliest-first:
                qk chunk d feeds attention pair d; v[bi] feeds batch bi."""
                def qk_em(dst, wname, d):
                    def em():
                        ps = psProj.tile([128, GB * T], FP32, tag="proj",
                                         name="psqk")
                        for k in range(3):
                            nc.tensor.matmul(
                                ps,
                                lhsT=w_sb[wname][:, k, d * 128:(d + 1) * 128],
                                rhs=xt[:, k, :, :],
                                start=(k == 0), stop=(k == 2),
                            )
                        nc.scalar.copy(dst[:, d, :, :], ps)
                    return em

                def v_em(bi, t2):
                    def em():
                        ps = psProj.tile([128, GB * T], FP32, tag="proj",
                                         name="psv")
                        for k in range(3):
                            nc.tensor.matmul(
                                ps[:, 0:C],
                                lhsT=xt[:, k, bi, t2 * 128:(t2 + 1) * 128],
                                rhs=w_sb["wv"][:, k, :],
                                start=(k == 0), stop=(k == 2),
                            )
                        nc.scalar.copy(vs[bi][:, t2, :], ps[:, 0:C])
                    return em

                return [
                    qk_em(qt, "wq", 0), qk_em(kt, "wk", 0),
                    v_em(0, 0), v_em(0, 1),
                    qk_em(qt, "wq", 1), qk_em(kt, "wk", 1),
                    qk_em(qt, "wq", 2), qk_em(kt, "wk", 2),
                    v_em(1, 0), v_em(1, 1),
                ]

            def att_pair(qt, kt, v, bi, pair, ot, filler=None):
                # ---- scores: fp32 psum, one bank per sub ----
                # col blocks per sub: 0 = (tq0,ts0), 1 = (tq1,ts0), 2 = (tq1,ts1)
                sc = [
                    psSc.tile([128, 3, 128], FP32, tag="sc", name=f"sc{s}")
                    for s in range(2)
                ]
                for s in range(2):
                    doff = s * 64
                    nc.tensor.matmul(
                        sc[s][:, 0, :],
                        lhsT=qt[doff:doff + 64, pair, bi, 0:128],
                        rhs=kt[doff:doff + 64, pair, bi, 0:128],
                        start=True, stop=True,
                    )
                for s in range(2):
                    doff = s * 64
                    nc.tensor.matmul(
                        sc[s][:, 1:3, :],
                        lhsT=qt[doff:doff + 64, pair, bi, 128:256],
                        rhs=kt[doff:doff + 64, pair, bi, 0:256],
                        start=True, stop=True,
                    )
                # ---- exp, then fused causal-mask * P + row sums ----
                p = pp.tile([128, 2, 3, 128], BF16, tag="p")
                sums = st.tile([128, 4], FP32, tag="sums")
                rs = st.tile([128, 4], FP32, tag="rs")
                for s in range(2):
                    nc.scalar.activation(
                        p[:, s, :, :], sc[s], AFT.Exp, scale=SCALE
                    )
                    nc.vector.scalar_tensor_tensor(
                        out=p[:, s, 0, :], in0=p[:, s, 0, :],
                        scalar=1.0, in1=mtri[:, 1, :],
                        op0=MUL, op1=MUL,
                        accum_out=sums[:, 2 * s:2 * s + 1],
                    )
                    nc.vector.scalar_tensor_tensor(
                        out=p[:, s, 1:3, :], in0=p[:, s, 1:3, :],
                        scalar=1.0, in1=mtri,
                        op0=MUL, op1=MUL,
                        accum_out=sums[:, 2 * s + 1:2 * s + 2],
                    )
                nc.vector.reciprocal(rs, sums)
                # ---- normalize P (DVE per-partition scalar mul) ----
                for s in range(2):
                    nc.vector.tensor_scalar_mul(
                        p[:, s, 0, :], p[:, s, 0, :], rs[:, 2 * s:2 * s + 1]
                    )
                    nc.vector.tensor_scalar_mul(
                        p[:, s, 1:3, :], p[:, s, 1:3, :],
                        rs[:, 2 * s + 1:2 * s + 2],
                    )
                if filler is not None:
                    filler()
                # ---- transpose P on the PE into one packed bank ----
                # piece order per sub: (ts0,tq0), (ts0,tq1), (ts1,tq1)
                ptp = psPt.tile([128, 2, 3, 128], BF16, tag="pt")
                pt = ptsb.tile([128, 2, 3, 128], BF16, tag="ptsb")
                for s in range(2):
                    nc.tensor.transpose(ptp[:, s, 0, :], p[:, s, 0, :], ident)
                    nc.tensor.transpose(ptp[:, s, 1, :], p[:, s, 1, :], ident)
                    nc.tensor.transpose(ptp[:, s, 2, :], p[:, s, 2, :], ident)
                nc.vector.tensor_copy(pt, ptp)
                # ---- AV: col-packed head pair ----
                po = psPo.tile([128, T], FP32, tag="po")
                for mm in range(3):
                    for s in range(2):
                        h = 2 * pair + s
                        doff = s * 64
                        vsl = lambda ts: v[:, ts, h * 64:(h + 1) * 64]
                        if mm == 0:
                            nc.tensor.matmul(
                                po[doff:doff + 64, 0:128],
                                lhsT=vsl(0), rhs=pt[:, s, 0, :],
                                start=True, stop=True,
                                tile_position=(0, doff),
                            )
                        else:
                            nc.tensor.matmul(
                                po[doff:doff + 64, 128:256],
                                lhsT=vsl(mm - 1), rhs=pt[:, s, mm, :],
                                start=(mm == 1), stop=(mm == 2),
                                tile_position=(0, doff),
                            )
                if pair == 1:
                    nc.scalar.copy(ot[:, pair, :], po)
                else:
                    nc.vector.tensor_copy(ot[:, pair, :], po)

            def emit_y(b, ot):
                ys = yp.tile([128, 2, C], BF16)
                for t2 in range(2):
                    ps = psProj.tile([128, GB * T], FP32, tag="proj", name="psy")
                    for k in range(3):
                        nc.tensor.matmul(
                            ps[:, 0:C],
                            lhsT=ot[:, k, t2 * 128:(t2 + 1) * 128],
                            rhs=w_sb["wo"][:, k, :],
                            start=(k == 0), stop=(k == 2),
                        )
                    nc.scalar.copy(ys[:, t2, :], ps[:, 0:C])
                nc.sync.dma_start(
                    out=y[b].rearrange("(t2 p) c -> p t2 c", p=128), in_=ys
                )

            # ---- prologue: only the chunks pair (b0, p0) needs up front;
            # the rest of group 0's projections become its own filler ----
            cur = load_group(0, xt=xt0)
            g0_ems = proj_emitters(cur[0], cur[1], cur[2], cur[3])
            for em in g0_ems[:4]:
                em()
            carry = g0_ems[4:]

            for g in range(NG):
                nxt = None
                nxt_ems = list(carry)
                carry = []
                if g + 1 < NG:
                    nxt = load_group(g + 1)
                    nxt_ems += proj_emitters(nxt[0], nxt[1], nxt[2], nxt[3])
                _, qt, kt, vs = cur
                ei = [0]

                def filler(nxt_ems=nxt_ems, ei=ei):
                    # one projection chunk, emitted between softmax and
                    # transposes so the PE has work during the DVE stages
                    if ei[0] < len(nxt_ems):
                        nxt_ems[ei[0]]()
                        ei[0] += 1

                for bi in range(GB):
                    b = g * GB + bi
                    ot = otp.tile([128, 3, T], BF16)
                    for pair in range(3):
                        filler()
                        att_pair(qt, kt, vs[bi], bi, pair, ot, filler=filler)
                    emit_y(b, ot)
                while ei[0] < len(nxt_ems):
                    nxt_ems[ei[0]]()
                    ei[0] += 1
                cur = nxt
    return nc


_NC = None


def _get_nc():
    global _NC
    if _NC is None:
        _NC = split_multi_waits(build_kernel())
    return _NC


def kernel(x, Wq, Wk, Wv, Wo, _trace=False):
    bf16 = ml_dtypes.bfloat16
    wq_t = np.ascontiguousarray(Wq.T).astype(bf16)
    wk_t = np.ascontiguousarray(Wk.T).astype(bf16)
    wv_t = np.ascontiguousarray(Wv.T).astype(bf16)
    wo_t = np.ascontiguousarray(Wo.T).astype(bf16)
    in_maps = []
    for i in range(N_CORES):
        xs = x[i * BL : (i + 1) * BL]  # [BL, T, C]
        xs_t = np.ascontiguousarray(xs.transpose(0, 2, 1)).astype(bf16)
        in_maps.append(
            {"xT": xs_t, "wqt": wq_t, "wkt": wk_t, "wvt": wv_t, "wot": wo_t}
        )
    res = run_bass_kernel_spmd(
        _get_nc(), in_maps, list(range(N_CORES)), trace=_trace
    )
    out = np.concatenate([r["y"] for r in res.results], axis=0)
    if _trace:
        return out.astype(np.float32), res
    return out.astype(np.float32)


# revision 24
# speedup vs baseline: 4.6444x; 1.0237x over previous
"""Causal multi-head attention (B=128, T=256, C=384, H=6, Dh=64) on 8 TRN2
NeuronCores, data-parallel over batch (16 batches per core, no collectives).

Layout strategy per core (v4 — software-pipelined):
  - host pre-transposes x to xT [b, C, T] and casts activations/weights to bf16
  - QT/KT computed as [D, T] (Dh on partitions) so scores = QT_h.T @ KT_h needs
    no on-chip transpose of Q/K; V computed as [T, D]
  - scores per head land in fp32 PSUM as three 128-col blocks
    [tq0ts0 | tq1ts1 | tq1ts0]
  - exp on Scalar; fused causal-mask-multiply + row-sum via one DVE
    scalar_tensor_tensor per (sub, tq-block); reciprocal + normalize on DVE
  - P transposed on the PE (bf16) into one packed PSUM bank per pair
  - AV col-packs the head pair via tile_position; output projection consumes
    OT [D, T] directly; y stored bf16 (host casts back to fp32)
  - group g+1's QK/V projection matmuls are emitted interleaved with group
    g's attention pairs so the PE always has filler work during softmax and
    the HAM clock gate stays warm
  - dedicated PSUM pools (proj 2 / scores 2 / transposes 2 / AV-out 2 banks)
"""

import sys

sys.path.insert(0, "/opt/trn_rl_repo")

import numpy as np
import ml_dtypes

import concourse.bass as bass
import concourse.tile as tile
from concourse import mybir
from concourse.bass_utils import run_bass_kernel_spmd
from concourse.masks import make_identity

def split_multi_waits(nc):
    """This walrus build accepts at most one sync-wait command per
    instruction; hoist extra waits into standalone InstEventSemaphore
    instructions on the same engine queue (queue waits run in order before
    the original instruction, so semantics are preserved)."""
    ctr = [0]

    def mk(engine, wait):
        ctr[0] += 1
        return mybir.InstEventSemaphore(
            name=f"WSPLIT-{ctr[0]}",
            engine=engine,
            ins=[],
            outs=[],
            sync_info=mybir.SyncInfo(on_wait=[wait], on_update=[]),
        )

    for f in nc.m.functions:
        for blk in f.blocks:
            insts = blk.instructions
            out = []
            for inst in insts:
                si = inst.sync_info
                if si is not None and len(si.on_wait) > 1:
                    waits = list(si.on_wait)
                    for w in waits[:-1]:
                        out.append(mk(inst.engine, w))
                    inst.sync_info = mybir.SyncInfo(
                        on_wait=[waits[-1]], on_update=list(si.on_update)
                    )
                out.append(inst)
            insts[:] = out
    return nc


N_CORES = 8
B, T, C = 128, 256, 384
H, DH = 6, 64
BL = B // N_CORES  # batches per core
GB = 2  # batches per projection group (N = GB*T = 512 <= one PSUM bank fp32)
NG = BL // GB
BF16 = mybir.dt.bfloat16
FP32 = mybir.dt.float32
AFT = mybir.ActivationFunctionType
MUL = mybir.AluOpType.mult
SCALE = DH**-0.5  # 0.125


def build_kernel() -> bass.Bass:
    nc = bass.Bass()
    xT = nc.dram_tensor("xT", [BL, C, T], BF16, kind="ExternalInput")
    wqt = nc.dram_tensor("wqt", [C, C], BF16, kind="ExternalInput")  # Wq.T [C, D]
    wkt = nc.dram_tensor("wkt", [C, C], BF16, kind="ExternalInput")
    wvt = nc.dram_tensor("wvt", [C, C], BF16, kind="ExternalInput")
    wot = nc.dram_tensor("wot", [C, C], BF16, kind="ExternalInput")  # Wo.T [D, C]
    y = nc.dram_tensor("y", [BL, T, C], BF16, kind="ExternalOutput")

    with tile.TileContext(nc) as tc:
        with (
            tc.tile_pool(name="const", bufs=1) as const,
            tc.tile_pool(name="xp", bufs=2) as xp,
            tc.tile_pool(name="qkv", bufs=2) as qkv,
            tc.tile_pool(name="pp", bufs=3) as pp,
            tc.tile_pool(name="st", bufs=3) as st,
            tc.tile_pool(name="ptsb", bufs=3) as ptsb,
            tc.tile_pool(name="otp", bufs=2) as otp,
            tc.tile_pool(name="yp", bufs=3) as yp,
            tc.tile_pool(name="psProj", bufs=2, space="PSUM") as psProj,
            tc.tile_pool(name="psSc", bufs=3, space="PSUM") as psSc,
            tc.tile_pool(name="psPt", bufs=2, space="PSUM") as psPt,
            tc.tile_pool(name="psPo", bufs=1, space="PSUM") as psPo,
        ):
            # prefetch x for group 0 ahead of the (larger) weight DMAs so the
            # first projections start ASAP
            xt0 = xp.tile([128, 3, GB, T], BF16, name="xt_g0")
            for bi in range(GB):
                nc.sync.dma_start(
                    out=xt0[:, :, bi, :],
                    in_=xT[bi].rearrange("(k p) t -> p k t", p=128),
                )
            ident = const.tile([128, 128], BF16)
            make_identity(nc, ident)
            # tiny dummy exp: forces the ACT exp-table load during the DMA
            # wait instead of on the first real softmax
            dummy = const.tile([128, 2], FP32)
            nc.scalar.activation(dummy, ident[:, 0:2], AFT.Exp, scale=1.0)
            # multiplicative causal masks (bf16), applied post-exp inside the
            # fused mask*P + row-sum DVE op. Block order per sub is
            # [tq0ts0 | tq1ts0 | tq1ts1]: blocks 0,2 are lower-triangle,
            # block 1 is all-ones. mtri covers blocks 1:3 (the tq1 row);
            # mtri[:,1,:] is reused for block 0.
            mtri = const.tile([128, 2, 128], BF16)
            nc.gpsimd.memset(mtri, 1.0)
            nc.gpsimd.affine_select(
                out=mtri[:, 1, :], in_=mtri[:, 1, :],
                compare_op=mybir.AluOpType.is_ge,
                fill=0.0, base=0, pattern=[[-1, 128]], channel_multiplier=1,
            )

            w_sb = {}
            for name, dram in (("wq", wqt), ("wk", wkt), ("wv", wvt), ("wo", wot)):
                w = const.tile([128, 3, C], BF16, tag=name)
                nc.sync.dma_start(out=w, in_=dram.rearrange("(k p) d -> p k d", p=128))
                w_sb[name] = w

            def load_group(g, xt=None):
                """DMA xT for group g, allocate qt/kt/v tiles."""
                if xt is None:
                    xt = xp.tile([128, 3, GB, T], BF16, name=f"xt{g}")
                    for bi in range(GB):
                        nc.sync.dma_start(
                            out=xt[:, :, bi, :],
                            in_=xT[g * GB + bi].rearrange(
                                "(k p) t -> p k t", p=128
                            ),
                        )
                qt = qkv.tile([128, 3, GB, T], BF16, tag="qt", name=f"qt{g}")
                kt = qkv.tile([128, 3, GB, T], BF16, tag="kt", name=f"kt{g}")
                vs = [
                    qkv.tile([128, 2, C], BF16, tag=f"v{bi}", name=f"v{g}_{bi}")
                    for bi in range(GB)
                ]
                return xt, qt, kt, vs

            def proj_emitters(xt, qt, kt, vs):
                """Closures each emitting one PSUM-chunk of the QK/V
                projections (3 accumulating matmuls + 1 evacuation). Ordered
                so the consumers' dependencies resolve earliest-first:
                qk chunk d feeds attention pair d; v[bi] feeds batch bi."""
                def qk_em(dst, wname, d):
                    def em():
                        ps = psProj.tile([128, GB * T], FP32, tag="proj",
                                         name="psqk")
                        for k in range(3):
                            nc.tensor.matmul(
                                ps,
                                lhsT=w_sb[wname][:, k, d * 128:(d + 1) * 128],
                                rhs=xt[:, k, :, :],
                                start=(k == 0), stop=(k == 2),
                            )
                        nc.scalar.copy(dst[:, d, :, :], ps)
                    return em

                def v_em(bi, t2):
                    def em():
                        ps = psProj.tile([128, GB * T], FP32, tag="proj",
                                         name="psv")
                        for k in range(3):
                            nc.tensor.matmul(
                                ps[:, 0:C],
                                lhsT=xt[:, k, bi, t2 * 128:(t2 + 1) * 128],
                                rhs=w_sb["wv"][:, k, :],
                                start=(k == 0), stop=(k == 2),
                            )
                        nc.scalar.copy(vs[bi][:, t2, :], ps[:, 0:C])
                    return em

                return [
                    qk_em(qt, "wq", 0), qk_em(kt, "wk", 0),
                    v_em(0, 0), v_em(0, 1),
                    qk_em(qt, "wq", 1), qk_em(kt, "wk", 1),
                    qk_em(qt, "wq", 2), qk_em(kt, "wk", 2),
                    v_em(1, 0), v_em(1, 1),
                ]

            po_state = {"tile": None, "idx": 0}

            def att_pair(qt, kt, v, bi, pair, ot, filler=None):
                # ---- scores: fp32 psum, one bank per sub ----
                # col blocks per sub: 0 = (tq0,ts0), 1 = (tq1,ts0), 2 = (tq1,ts1)
                sc = [
                    psSc.tile([128, 3, 128], FP32, tag="sc", name=f"sc{s}")
                    for s in range(2)
                ]
                for s in range(2):
                    doff = s * 64
                    nc.tensor.matmul(
                        sc[s][:, 0, :],
                        lhsT=qt[doff:doff + 64, pair, bi, 0:128],
                        rhs=kt[doff:doff + 64, pair, bi, 0:128],
                        start=True, stop=True,
                    )
                for s in range(2):
                    doff = s * 64
                    nc.tensor.matmul(
                        sc[s][:, 1:3, :],
                        lhsT=qt[doff:doff + 64, pair, bi, 128:256],
                        rhs=kt[doff:doff + 64, pair, bi, 0:256],
                        start=True, stop=True,
                    )
                # ---- exp, then fused causal-mask * P + row sums ----
                p = pp.tile([128, 2, 3, 128], BF16, tag="p")
                sums = st.tile([128, 4], FP32, tag="sums")
                rs = st.tile([128, 4], FP32, tag="rs")
                for s in range(2):
                    nc.scalar.activation(
                        p[:, s, :, :], sc[s], AFT.Exp, scale=SCALE
                    )
                    nc.vector.scalar_tensor_tensor(
                        out=p[:, s, 0, :], in0=p[:, s, 0, :],
                        scalar=1.0, in1=mtri[:, 1, :],
                        op0=MUL, op1=MUL,
                        accum_out=sums[:, 2 * s:2 * s + 1],
                    )
                    nc.vector.scalar_tensor_tensor(
                        out=p[:, s, 1:3, :], in0=p[:, s, 1:3, :],
                        scalar=1.0, in1=mtri,
                        op0=MUL, op1=MUL,
                        accum_out=sums[:, 2 * s + 1:2 * s + 2],
                    )
                nc.vector.reciprocal(rs, sums)
                # ---- normalize P (DVE per-partition scalar mul) ----
                for s in range(2):
                    nc.vector.tensor_scalar_mul(
                        p[:, s, 0, :], p[:, s, 0, :], rs[:, 2 * s:2 * s + 1]
                    )
                    nc.vector.tensor_scalar_mul(
                        p[:, s, 1:3, :], p[:, s, 1:3, :],
                        rs[:, 2 * s + 1:2 * s + 2],
                    )
                if filler is not None:
                    filler()
                # ---- transpose P on the PE into one packed bank ----
                # piece order per sub: (ts0,tq0), (ts0,tq1), (ts1,tq1)
                ptp = psPt.tile([128, 2, 3, 128], BF16, tag="pt")
                pt = ptsb.tile([128, 2, 3, 128], BF16, tag="ptsb")
                for s in range(2):
                    nc.tensor.transpose(ptp[:, s, 0, :], p[:, s, 0, :], ident)
                    nc.tensor.transpose(ptp[:, s, 1, :], p[:, s, 1, :], ident)
                    nc.tensor.transpose(ptp[:, s, 2, :], p[:, s, 2, :], ident)
                nc.vector.tensor_copy(pt, ptp)
                # ---- AV: col-packed head pair; two pairs share one PSUM
                # bank ([128,512] fp32), alternating halves ----
                if po_state["idx"] % 2 == 0:
                    po_state["tile"] = psPo.tile(
                        [128, 2, T], FP32, tag="po", name="po2"
                    )
                po = po_state["tile"][:, po_state["idx"] % 2, :]
                po_state["idx"] += 1
                for mm in range(3):
                    for s in range(2):
                        h = 2 * pair + s
                        doff = s * 64
                        vsl = lambda ts: v[:, ts, h * 64:(h + 1) * 64]
                        if mm == 0:
                            nc.tensor.matmul(
                                po[doff:doff + 64, 0:128],
                                lhsT=vsl(0), rhs=pt[:, s, 0, :],
                                start=True, stop=True,
                                tile_position=(0, doff),
                            )
                        else:
                            nc.tensor.matmul(
                                po[doff:doff + 64, 128:256],
                                lhsT=vsl(mm - 1), rhs=pt[:, s, mm, :],
                                start=(mm == 1), stop=(mm == 2),
                                tile_position=(0, doff),
                            )
                if pair == 1:
                    nc.scalar.copy(ot[:, pair, :], po)
                else:
                    nc.vector.tensor_copy(ot[:, pair, :], po)

            def emit_y(b, ot):
                ys = yp.tile([128, 2, C], BF16)
                for t2 in range(2):
                    ps = psProj.tile([128, GB * T], FP32, tag="proj", name="psy")
                    for k in range(3):
                        nc.tensor.matmul(
                            ps[:, 0:C],
                            lhsT=ot[:, k, t2 * 128:(t2 + 1) * 128],
                            rhs=w_sb["wo"][:, k, :],
                            start=(k == 0), stop=(k == 2),
                        )
                    nc.scalar.copy(ys[:, t2, :], ps[:, 0:C])
                nc.sync.dma_start(
                    out=y[b].rearrange("(t2 p) c -> p t2 c", p=128), in_=ys
                )

            # ---- prologue: only the chunks pair (b0, p0) needs up front;
            # the rest of group 0's projections become its own filler ----
            cur = load_group(0, xt=xt0)
            g0_ems = proj_emitters(cur[0], cur[1], cur[2], cur[3])
            for em in g0_ems[:4]:
                em()
            carry = g0_ems[4:]

            for g in range(NG):
                nxt = None
                nxt_ems = list(carry)
                carry = []
                if g + 1 < NG:
                    nxt = load_group(g + 1)
                    nxt_ems += proj_emitters(nxt[0], nxt[1], nxt[2], nxt[3])
                _, qt, kt, vs = cur
                ei = [0]

                def filler(nxt_ems=nxt_ems, ei=ei):
                    # one projection chunk, emitted between softmax and
                    # transposes so the PE has work during the DVE stages
                    if ei[0] < len(nxt_ems):
                        nxt_ems[ei[0]]()
                        ei[0] += 1

                for bi in range(GB):
                    b = g * GB + bi
                    ot = otp.tile([128, 3, T], BF16)
                    for pair in range(3):
                        filler()
                        att_pair(qt, kt, vs[bi], bi, pair, ot, filler=filler)
                    emit_y(b, ot)
                while ei[0] < len(nxt_ems):
                    nxt_ems[ei[0]]()
                    ei[0] += 1
                cur = nxt
    return nc


_NC = None


def _get_nc():
    global _NC
    if _NC is None:
        _NC = split_multi_waits(build_kernel())
    return _NC


def kernel(x, Wq, Wk, Wv, Wo, _trace=False):
    bf16 = ml_dtypes.bfloat16
    wq_t = np.ascontiguousarray(Wq.T).astype(bf16)
    wk_t = np.ascontiguousarray(Wk.T).astype(bf16)
    wv_t = np.ascontiguousarray(Wv.T).astype(bf16)
    wo_t = np.ascontiguousarray(Wo.T).astype(bf16)
    in_maps = []
    for i in range(N_CORES):
        xs = x[i * BL : (i + 1) * BL]  # [BL, T, C]
        xs_t = np.ascontiguousarray(xs.transpose(0, 2, 1)).astype(bf16)
        in_maps.append(
            {"xT": xs_t, "wqt": wq_t, "wkt": wk_t, "wvt": wv_t, "wot": wo_t}
        )
    res = run_bass_kernel_spmd(
        _get_nc(), in_maps, list(range(N_CORES)), trace=_trace
    )
    out = np.concatenate([r["y"] for r in res.results], axis=0)
    if _trace:
        return out.astype(np.float32), res
    return out.astype(np.float32)
